# revision 1
# baseline (speedup 1.0000x reference)
"""GCN encoder (2-layer GCNConv + global mean pool) on 8 Trainium2 NeuronCores.

Strategy (graph/data parallel, per the sharding hint):
- Nodes partitioned into 8 contiguous blocks; each core owns its nodes' in-edges.
- GCN normalization factors: agg_d = dinv_d * (sum_e dinv_src*x_src + dinv_d*x_d)
  and the dense W matmul commutes with the (linear) aggregation, so each layer:
    launch computes t = x*dinv once (node-major, per-partition scale),
    host expands t by edge source into dst-sorted feature-major columns
    (np.take only - index-driven movement, zero host float math),
    device does a feature-major DVE segmented reduction (uniform-degree
    buckets), adds the self-loop row, applies W, the outer dinv scale,
    bias and relu on-chip.
- The host expansion between launches doubles as the halo exchange the
  sharding hint calls for. Pooling partial sums + per-graph counts are
  combined with an 8-core AllReduce; the mean division happens on-chip.
"""
import sys
sys.path.insert(0, "/opt/trn_rl_repo")

import numpy as np
import ml_dtypes

import concourse.bass as bass
import concourse.bacc as bacc
import concourse.mybir as mybir
import concourse.tile as tile
from concourse.bass_utils import run_bass_kernel_spmd

NCORES = 8
P = 128
N_NODES = 50000
IN_DIM = 128
HID_DIM = 128
OUT_DIM = 64
N_GRAPHS = 64

OWN = N_NODES // NCORES
CHUNK = 8192
N_PAD = -(-N_NODES // P) * P      # 50048
GTILE = N_PAD // P                # 391

BF16 = mybir.dt.bfloat16
F32 = mybir.dt.float32


def _ceil(a, b):
    return -(-a // b) * b


# ----------------------------------------------------------------- host prep
def host_prep(edge_index, batch):
    src = np.asarray(edge_index[0], dtype=np.int64)
    dst = np.asarray(edge_index[1], dtype=np.int64)
    batch = np.asarray(batch, dtype=np.int64)

    deg = np.bincount(dst, minlength=N_NODES) + 1

    cores = []
    for c in range(NCORES):
        lo, hi = c * OWN, (c + 1) * OWN
        mask = (dst >= lo) & (dst < hi)
        e_src = src[mask]
        e_dst = dst[mask] - lo
        order = np.argsort(e_dst, kind="stable")
        e_src = e_src[order]
        kdeg = np.bincount(e_dst[order], minlength=OWN)
        cores.append({"e_src": e_src, "kdeg": kdeg})

    all_k = sorted(set().union(*[set(np.unique(c["kdeg"])) for c in cores]) - {0})
    bucket_n = {k: max(int((c["kdeg"] == k).sum()) for c in cores) for k in all_k}
    zero_max = max(int((c["kdeg"] == 0).sum()) for c in cores)

    own_pad = _ceil(zero_max + sum(bucket_n.values()), P)
    ntile = own_pad // P

    pieces = []
    chunk_used, cur_chunk, agg_col = 0, 0, zero_max
    for k in all_k:
        n_b, done = bucket_n[k], 0
        while done < n_b:
            fit = min(n_b - done, (CHUNK - chunk_used) // k)
            # split at 128-aggcol boundaries so each piece writes one agg tile
            fit = min(fit, P - (agg_col % P)) if fit else fit
            if fit == 0:
                chunk_used = 0
                cur_chunk += 1
                continue
            pieces.append((cur_chunk, chunk_used, fit, k, agg_col))
            chunk_used += fit * k
            agg_col += fit
            done += fit
    n_chunks = cur_chunk + (1 if chunk_used > 0 else 0)
    total_cols = n_chunks * CHUNK

    per_core = []
    for c in range(NCORES):
        kdeg, e_src = cores[c]["kdeg"], cores[c]["e_src"]
        offs = np.zeros(OWN + 1, np.int64)
        np.cumsum(kdeg, out=offs[1:])
        nodes_by_k = {k: np.where(kdeg == k)[0] for k in all_k}
        used = {k: 0 for k in all_k}
        slot_src = np.full(total_cols, -1, np.int64)
        full_map = np.full(own_pad, -1, np.int64)
        zn = np.where(kdeg == 0)[0]
        full_map[:len(zn)] = zn
        for (chunk, cstart, n_n, k, acol) in pieces:
            base = chunk * CHUNK + cstart
            nodes = nodes_by_k[k][used[k]:used[k] + n_n]
            used[k] += n_n
            nn = len(nodes)
            if nn > 0:
                idx = (offs[nodes][:, None] + np.arange(k)[None, :]).ravel()
                cols = (base + (np.arange(nn)[:, None] * k
                                + np.arange(k)[None, :])).ravel()
                slot_src[cols] = e_src[idx]
                full_map[acol:acol + nn] = nodes
        per_core.append({"slot_src": slot_src, "full_map": full_map})

    onehots, deg_own_w = [], []
    for c in range(NCORES):
        lo = c * OWN
        fm = per_core[c]["full_map"]
        real = fm >= 0
        oh = np.zeros((own_pad, N_GRAPHS), np.float32)
        oh[np.where(real)[0], batch[lo + fm[real]]] = 1.0
        onehots.append(np.ascontiguousarray(oh.reshape(ntile, P, N_GRAPHS).transpose(1, 0, 2)))
        d = np.ones(own_pad, np.float32)
        d[real] = deg[lo + fm[real]]
        # wrapped: [P, ntile], node (t*P+p) -> [p, t]
        deg_own_w.append(np.ascontiguousarray(d.reshape(ntile, P).T))

    dg = np.ones(N_PAD, np.float32)
    dg[:N_NODES] = deg
    deg_g_w = np.ascontiguousarray(dg.reshape(GTILE, P).T)  # [P, GTILE]

    return {
        "pieces": pieces, "n_chunks": n_chunks, "total_cols": total_cols,
        "per_core": per_core, "onehots": onehots, "deg_own_w": deg_own_w,
        "deg_g_w": deg_g_w, "own_pad": own_pad, "ntile": ntile,
    }


def expand_T(table_bf, prep):
    """Node-major [total_cols, F] expansion; device transposes via DMA xbar."""
    nz = np.zeros((1, table_bf.shape[1]), dtype=table_bf.dtype)
    tz = np.concatenate([table_bf, nz], axis=0)
    out = []
    for c in range(NCORES):
        ss = prep["per_core"][c]["slot_src"]
        ssc = np.where(ss >= 0, ss, table_bf.shape[0])
        out.append(tz[ssc])
    return out


def own_T(table_bf, prep, c):
    fm = prep["per_core"][c]["full_map"]
    lo = c * OWN
    e = np.zeros((prep["own_pad"], table_bf.shape[1]), dtype=ml_dtypes.bfloat16)
    real = fm >= 0
    e[real] = table_bf[lo + fm[real]]
    return np.ascontiguousarray(e.T)


# --------------------------------------------------------------- bass builders
def build_scale(prep):
    """launch-0: t = x * rsqrt(deg), node-major, replicated on all cores."""
    nc = bacc.Bacc("TRN2", target_bir_lowering=False, debug=False,
                   num_devices=NCORES)
    x_in = nc.dram_tensor("x", [N_PAD, IN_DIM], F32, kind="ExternalInput")
    dg = nc.dram_tensor("dg", [P, GTILE], F32, kind="ExternalInput")
    out = nc.dram_tensor("out", [N_PAD, IN_DIM], BF16, kind="ExternalOutput")
    with tile.TileContext(nc) as tc:
        with (
            tc.tile_pool(name="c", bufs=1) as cp,
            tc.tile_pool(name="x", bufs=4) as xp,
        ):
            dt_ = cp.tile([P, GTILE], F32)
            nc.sync.dma_start(out=dt_[:], in_=dg[:])
            dinv = cp.tile([P, GTILE], F32)
            nc.scalar.sqrt(dinv[:], dt_[:])
            nc.vector.reciprocal(dinv[:], dinv[:])
            for t in range(GTILE):
                xt = xp.tile([P, IN_DIM], F32, tag="x")
                nc.sync.dma_start(out=xt[:], in_=x_in[t * P:(t + 1) * P, :])
                ot = xp.tile([P, IN_DIM], BF16, tag="o")
                nc.scalar.activation(ot[:], xt[:],
                                     mybir.ActivationFunctionType.Copy,
                                     bias=0.0, scale=dinv[:, t:t + 1])
                nc.sync.dma_start(out=out[t * P:(t + 1) * P, :], in_=ot[:])
    nc.compile()
    return nc


def build_layer(prep, fdim, odim, pool=False, rep=1):
    n_chunks, total_cols = prep["n_chunks"], prep["total_cols"]
    own_pad, ntile = prep["own_pad"], prep["ntile"]
    pieces = prep["pieces"]

    nc = bacc.Bacc("TRN2", target_bir_lowering=False, debug=False,
                   num_devices=NCORES)
    x_exp = nc.dram_tensor("x_exp", [total_cols, fdim], BF16, kind="ExternalInput")
    x_own = nc.dram_tensor("x_own", [fdim, own_pad], BF16, kind="ExternalInput")
    down = nc.dram_tensor("down", [P, ntile], F32, kind="ExternalInput")
    W = nc.dram_tensor("W", [fdim, odim], F32, kind="ExternalInput")
    b = nc.dram_tensor("b", [1, odim], F32, kind="ExternalInput")
    if pool:
        oh_in = nc.dram_tensor("onehot", [P, ntile, N_GRAPHS], F32,
                               kind="ExternalInput")
        out = nc.dram_tensor("out", [N_GRAPHS, OUT_DIM], F32, kind="ExternalOutput")
        ar_in = nc.dram_tensor("ar_in", [N_GRAPHS, N_GRAPHS + 1], F32)
        ar_out = nc.dram_tensor("ar_out", [N_GRAPHS, N_GRAPHS + 1], F32,
                                addr_space="Shared")
    else:
        out = nc.dram_tensor("out", [own_pad, odim], F32, kind="ExternalOutput")

    from concourse.masks import make_identity

    with tile.TileContext(nc) as tc:
        with (
            tc.tile_pool(name="const", bufs=1) as cp,
            tc.tile_pool(name="xc", bufs=4) as xp,
            tc.tile_pool(name="ps", bufs=2, space="PSUM") as pp,
            tc.tile_pool(name="ps2", bufs=1, space="PSUM") as pp2,
            tc.tile_pool(name="sm", bufs=3) as sp,
        ):
            Wt = cp.tile([fdim, odim], F32)
            nc.sync.dma_start(out=Wt[:], in_=W[:])
            ones_full = cp.tile([P, P], F32)
            nc.vector.memset(ones_full[:], 1.0)
            ones_row = ones_full[0:1, :]
            ident = cp.tile([P, P], F32)
            make_identity(nc, ident[:])
            if pool:
                oht = cp.tile([P, ntile, N_GRAPHS], F32)
                nc.sync.dma_start(out=oht[:], in_=oh_in[:])

            # bias broadcast [P, odim]
            brow_full = cp.tile([P, odim], F32)
            nc.sync.dma_start(out=brow_full[0:1, :], in_=b[:])
            bp = pp.tile([P, odim], F32, tag="bb")
            nc.tensor.matmul(bp[:], ones_row, brow_full[0:1, :], start=True, stop=True)
            biasb = cp.tile([P, odim], F32)
            nc.scalar.copy(biasb[:], bp[:])

            xot = cp.tile([fdim, own_pad], BF16)
            nc.sync.dma_start(out=xot[:], in_=x_own[:])
            xof = cp.tile([fdim, own_pad], F32)
            nc.vector.tensor_copy(out=xof[:], in_=xot[:])

            dw = cp.tile([P, ntile], F32)
            nc.sync.dma_start(out=dw[:], in_=down[:])
            dinv = cp.tile([P, ntile], F32)
            nc.scalar.sqrt(dinv[:], dw[:])
            nc.vector.reciprocal(dinv[:], dinv[:])

            agg_t = []
            for t in range(ntile):
                a = cp.tile([P, P], F32, tag=f"agg{t}")
                nc.vector.memset(a[:], 0.0)
                agg_t.append(a)

            by_chunk = [[] for _ in range(n_chunks)]
            for pc in pieces:
                by_chunk[pc[0]].append(pc)

            for _rep in range(rep):
                for ch in range(n_chunks):
                    xt = xp.tile([fdim, CHUNK], BF16, tag="xc")
                    nc.sync.dma_start_transpose(
                        out=xt[:], in_=x_exp[ch * CHUNK:(ch + 1) * CHUNK, :])
                    for (_, cstart, n_n, k, acol) in by_chunk[ch]:
                        at, ac = agg_t[acol // P], acol % P
                        nc.vector.tensor_reduce(
                            out=at[:, ac:ac + n_n],
                            in_=xt[:, cstart:cstart + n_n * k].rearrange(
                                "p (n k) -> p n k", k=k),
                            axis=mybir.AxisListType.X, op=mybir.AluOpType.add,
                        )


                if pool:
                    pps = pp2.tile([N_GRAPHS, N_GRAPHS + 1], F32, tag="pool")
                for t in range(ntile):
                    it = sp.tile([P, P], F32, tag="inner")
                    nc.vector.tensor_add(out=it[:], in0=agg_t[t][:],
                                         in1=xof[:, t * P:(t + 1) * P])
                    # node-major matmul: lhsT = inner tile (stationary), rhs = W
                    zp = pp.tile([P, odim], F32, tag="z")
                    nc.tensor.matmul(zp[:], it[:], Wt[:], start=True, stop=True)
                    if pool:
                        hn = sp.tile([P, odim + 1], F32, tag="hn")
                        nc.vector.memset(hn[:, odim:odim + 1], 1.0)
                        # h = relu(dinv*z + bias), fused scale+bias on DVE
                        nc.vector.scalar_tensor_tensor(
                            out=hn[:, :odim], in0=zp[:], scalar=dinv[:, t:t + 1],
                            in1=biasb[:], op0=mybir.AluOpType.mult,
                            op1=mybir.AluOpType.add)
                        nc.vector.tensor_relu(out=hn[:, :odim], in_=hn[:, :odim])
                        nc.tensor.matmul(pps[:], oht[:, t, :], hn[:],
                                         start=(t == 0), stop=(t == ntile - 1))
                    else:
                        hr = sp.tile([P, odim], F32, tag="hr")
                        nc.vector.scalar_tensor_tensor(
                            out=hr[:], in0=zp[:], scalar=dinv[:, t:t + 1],
                            in1=biasb[:], op0=mybir.AluOpType.mult,
                            op1=mybir.AluOpType.add)
                        nc.vector.tensor_relu(out=hr[:], in_=hr[:])
                        # output h * dinv (pre-scaled table for next layer)
                        hs = sp.tile([P, odim], F32, tag="hs")
                        nc.scalar.activation(hs[:], hr[:],
                                             mybir.ActivationFunctionType.Copy,
                                             bias=0.0, scale=dinv[:, t:t + 1])
                        nc.sync.dma_start(out=out[t * P:(t + 1) * P, :], in_=hs[:])

            if pool:
                pool_sb = cp.tile([N_GRAPHS, N_GRAPHS + 1], F32)
                nc.scalar.copy(pool_sb[:], pps[:])
                nc.gpsimd.dma_start(out=ar_in[:], in_=pool_sb[:])
                nc.gpsimd.collective_compute(
                    "AllReduce", mybir.AluOpType.add,
                    replica_groups=[list(range(NCORES))],
                    ins=[ar_in[:]], outs=[ar_out[:]],
                )
                red = cp.tile([N_GRAPHS, N_GRAPHS + 1], F32)
                nc.sync.dma_start(out=red[:], in_=ar_out[:])
                cnt = cp.tile([N_GRAPHS, 1], F32)
                nc.vector.tensor_scalar_max(out=cnt[:],
                                            in0=red[:, N_GRAPHS:N_GRAPHS + 1],
                                            scalar1=1.0)
                nc.vector.reciprocal(cnt[:], cnt[:])
                res = cp.tile([N_GRAPHS, OUT_DIM], F32)
                nc.scalar.activation(res[:], red[:, :OUT_DIM],
                                     mybir.ActivationFunctionType.Copy,
                                     bias=0.0, scale=cnt[:])
                nc.sync.dma_start(out=out[:], in_=res[:])
    nc.compile()
    return nc


# --------------------------------------------------------------------- kernel
_cache = {}


def run_gcn(x, W1, b1, W2, b2, edge_index, batch, num_graphs, rep=1):
    x = np.asarray(x, dtype=np.float32)
    W1 = np.asarray(W1, dtype=np.float32)
    b1 = np.asarray(b1, dtype=np.float32).reshape(1, -1)
    W2 = np.asarray(W2, dtype=np.float32)
    b2 = np.asarray(b2, dtype=np.float32).reshape(1, -1)

    ei = np.asarray(edge_index)
    ba = np.asarray(batch)
    key = (rep, int(ei[0, :64].sum()), int(ei[1, -64:].sum()), int(ba[:512].sum()))
    if key not in _cache:
        prep = host_prep(edge_index, batch)
        nc0 = build_scale(prep)
        nc1 = build_layer(prep, IN_DIM, HID_DIM, pool=False, rep=rep)
        nc2 = build_layer(prep, HID_DIM, OUT_DIM, pool=True, rep=rep)
        _cache[key] = (prep, nc0, nc1, nc2)
    prep, nc0, nc1, nc2 = _cache[key]

    xpad = np.zeros((N_PAD, IN_DIM), np.float32)
    xpad[:N_NODES] = x
    in0 = [{"x": xpad, "dg": prep["deg_g_w"]}] * NCORES
    r0 = run_bass_kernel_spmd(nc0, in0, core_ids=list(range(NCORES)))
    t1 = r0.results[0]["out"][:N_NODES]  # x*dinv, bf16

    t1_exps = expand_T(t1, prep)
    in1 = [{
        "x_exp": t1_exps[c], "x_own": own_T(t1, prep, c),
        "down": prep["deg_own_w"][c], "W": W1, "b": b1,
    } for c in range(NCORES)]
    r1 = run_bass_kernel_spmd(nc1, in1, core_ids=list(range(NCORES)))

    # hs = h*dinv per core, reassemble to global table (bf16 for expansion)
    hs = np.zeros((N_NODES, HID_DIM), np.float32)
    for c in range(NCORES):
        fm = prep["per_core"][c]["full_map"]
        real = fm >= 0
        hs[c * OWN + fm[real]] = r1.results[c]["out"][np.where(real)[0]]
    hsb = hs.astype(ml_dtypes.bfloat16)

    hs_exps = expand_T(hsb, prep)
    in2 = [{
        "x_exp": hs_exps[c], "x_own": own_T(hsb, prep, c),
        "down": prep["deg_own_w"][c], "W": W2, "b": b2,
        "onehot": prep["onehots"][c],
    } for c in range(NCORES)]
    r2 = run_bass_kernel_spmd(nc2, in2, core_ids=list(range(NCORES)))
    return r2.results[0]["out"][:int(num_graphs), :].copy()


def kernel(x, W1, b1, W2, b2, edge_index, batch, num_graphs):
    return run_gcn(x, W1, b1, W2, b2, edge_index, batch, num_graphs, rep=1)



# revision 8
# speedup vs baseline: 78.7562x; 78.7562x over previous
"""GCN encoder (2-layer GCNConv + global mean pool) on 8 Trainium2 NeuronCores.

Strategy (graph/data parallel per the sharding hint):
- Nodes partitioned into 8 blocks of 6250; within each core, nodes are
  permuted by in-degree (descending) so the 49 node-tiles of 128 have
  near-uniform message counts.
- Self-loops become explicit messages (slot 0 of every node), so each
  GCN layer is: gather message rows from the node table with
  qPoolDynamic indirect DMAs (one 128-row gather per slot), a strided
  DVE tensor_reduce over slots, a PE transpose, and the dense W matmul
  with bias/relu/dinv scaling fused on-chip.
- Three SPMD launches share device-resident bf16 node tables:
  stage0 computes t0 = x*dinv and AllGathers the full table; stage1
  computes layer 1 into table t1 (AllGather); stage2 computes layer 2,
  pools with an on-chip one-hot matmul, and AllReduces per-graph sums
  and counts across the 8 cores before the mean division.
- The launches go through a cached jax.jit(shard_map(bass_exec)) per
  stage (the same PJRT path run_bass_kernel_spmd uses under axon), so
  warm calls only ship x (bf16) and fetch the [64, 64] result; tables,
  indices and weights stay on the devices.
"""
import sys
sys.path.insert(0, "/opt/trn_rl_repo")

import functools

import numpy as np
import ml_dtypes
import jax
import jax.numpy as jnp
from jax.experimental.shard_map import shard_map
from jax.sharding import Mesh, PartitionSpec, NamedSharding

import concourse.bass as bass
import concourse.bacc as bacc
import concourse.mybir as mybir
import concourse.tile as tile
from concourse.bass2jax import (_bass_exec_p, install_neuronx_cc_hook,
                                partition_id_tensor)
from concourse.masks import make_identity

NCORES = 8
P = 128
N_NODES = 50000
OWN = N_NODES // NCORES          # 6250
NT = 49                          # node tiles per core
OWN_PAD = NT * P                 # 6272
R_TOT = NCORES * OWN_PAD         # 50176
IN_DIM = 128
HID_DIM = 128
OUT_DIM = 64
N_GRAPHS = 64

BF16 = mybir.dt.bfloat16
F32 = mybir.dt.float32
I32 = mybir.dt.int32
Copy = mybir.ActivationFunctionType.Copy


# ----------------------------------------------------------------- host prep
def host_prep(edge_index, batch):
    src = np.asarray(edge_index[0]).astype(np.int64)
    dst = np.asarray(edge_index[1]).astype(np.int64)
    bat = np.asarray(batch).astype(np.int64)
    deg = np.bincount(dst, minlength=N_NODES).astype(np.int64) + 1  # A+I degree

    order = np.empty(N_NODES, np.int64)   # permuted order: global node ids
    row_of = np.empty(N_NODES, np.int64)  # node id -> table row
    for c in range(NCORES):
        dg = deg[c * OWN:(c + 1) * OWN]
        o = np.argsort(-dg, kind="stable")
        order[c * OWN:(c + 1) * OWN] = c * OWN + o
        pos = np.empty(OWN, np.int64)
        pos[o] = np.arange(OWN)
        row_of[c * OWN:(c + 1) * OWN] = c * OWN_PAD + pos

    deg_perm = deg[order]
    kt = np.zeros((NCORES, NT), np.int64)
    for c in range(NCORES):
        dp = deg_perm[c * OWN:(c + 1) * OWN]
        for t in range(NT):
            kt[c, t] = dp[t * P] if t * P < OWN else 1
    kmax = np.maximum(kt.max(axis=0), 1)                 # [NT] shared
    koff = np.zeros(NT + 1, np.int64)
    np.cumsum(kmax, out=koff[1:])
    ktot = int(koff[-1])

    # message source rows: slot 0 = self, slots 1.. = in-edges, rest dummy.
    # dummy points at each core's first pad row, which is exactly zero.
    idx = np.empty((NCORES, P, ktot), np.int32)
    for c in range(NCORES):
        idx[c, :, :] = c * OWN_PAD + OWN
        pos = np.arange(OWN)
        t_n, p_n = pos // P, pos % P
        idx[c, p_n, koff[t_n]] = (c * OWN_PAD + pos).astype(np.int32)

    rd = row_of[dst]
    rs = row_of[src].astype(np.int32)
    eorder = np.argsort(rd, kind="stable")
    rd_s, rs_s = rd[eorder], rs[eorder]
    grp = np.flatnonzero(np.r_[True, rd_s[1:] != rd_s[:-1]])
    sizes = np.diff(np.r_[grp, len(rd_s)])
    cc = np.arange(len(rd_s)) - np.repeat(grp, sizes)
    c_e = rd_s // OWN_PAD
    p_loc = rd_s % OWN_PAD
    t_e, p_e = p_loc // P, p_loc % P
    idx[c_e, p_e, koff[t_e] + cc + 1] = rs_s

    # wrapped per-node tables [core, 128, NT]; pads: deg=inf (dinv=0), batch=-1
    degw = np.full((NCORES, P, NT), np.inf, np.float32)
    batw = np.full((NCORES, P, NT), -1.0, np.float32)
    for c in range(NCORES):
        d = np.full(OWN_PAD, np.inf, np.float32)
        d[:OWN] = deg_perm[c * OWN:(c + 1) * OWN]
        b = np.full(OWN_PAD, -1.0, np.float32)
        b[:OWN] = bat[order[c * OWN:(c + 1) * OWN]]
        degw[c] = d.reshape(NT, P).T
        batw[c] = b.reshape(NT, P).T

    iota64 = np.broadcast_to(
        np.arange(N_GRAPHS, dtype=np.float32), (P, N_GRAPHS)).copy()
    return {"order": order, "kmax": kmax, "koff": koff, "ktot": ktot,
            "idx": idx, "degw": degw, "batw": batw, "iota64": iota64}


def stage_x(x, prep):
    """Permute x into table order, pad with zeros, cast bf16."""
    xg = np.zeros((NCORES * OWN_PAD, IN_DIM), ml_dtypes.bfloat16)
    xperm = x[prep["order"]]
    for c in range(NCORES):
        xg[c * OWN_PAD:c * OWN_PAD + OWN] = xperm[c * OWN:(c + 1) * OWN]
    return xg


# --------------------------------------------------------------- bass stages
def build_stage0():
    nc = bacc.Bacc("TRN2", target_bir_lowering=False, debug=False,
                   num_devices=NCORES)
    x = nc.dram_tensor("x", [OWN_PAD, IN_DIM], BF16, kind="ExternalInput")
    degt = nc.dram_tensor("degw", [P, NT], F32, kind="ExternalInput")
    T0 = nc.dram_tensor("T0", [R_TOT, IN_DIM], BF16, kind="ExternalOutput")
    T0_in = nc.dram_tensor("T0_in", [OWN_PAD, IN_DIM], BF16)
    T0_g = nc.dram_tensor("T0_g", [R_TOT, IN_DIM], BF16, addr_space="Shared")
    with tile.TileContext(nc) as tc:
        with (
            tc.tile_pool(name="c", bufs=1) as cp,
            tc.tile_pool(name="x", bufs=4) as xp,
        ):
            dw = cp.tile([P, NT], F32)
            nc.sync.dma_start(out=dw[:], in_=degt[:])
            dinv = cp.tile([P, NT], F32)
            nc.scalar.sqrt(dinv[:], dw[:])
            nc.vector.reciprocal(dinv[:], dinv[:])
            for t in range(NT):
                xt = xp.tile([P, IN_DIM], BF16, tag="x")
                nc.sync.dma_start(out=xt[:], in_=x[t * P:(t + 1) * P, :])
                ot = xp.tile([P, IN_DIM], BF16, tag="o")
                nc.scalar.activation(ot[:], xt[:], Copy,
                                     bias=0.0, scale=dinv[:, t:t + 1])
                nc.sync.dma_start(out=T0_in[t * P:(t + 1) * P, :], in_=ot[:])
            nc.gpsimd.collective_compute(
                "AllGather", mybir.AluOpType.bypass,
                replica_groups=[list(range(NCORES))],
                ins=[T0_in[:]], outs=[T0_g[:]])
            nc.sync.dma_start(out=T0[:], in_=T0_g[:])
    nc.compile()
    return nc


def build_layer(prep, fdim, odim, pool):
    kmax, koff, ktot = prep["kmax"], prep["koff"], prep["ktot"]
    km_cap = int(kmax.max())

    nc = bacc.Bacc("TRN2", target_bir_lowering=False, debug=False,
                   num_devices=NCORES)
    Tin = nc.dram_tensor("Tin", [R_TOT, fdim], BF16, kind="ExternalInput")
    idxd = nc.dram_tensor("idx", [P, ktot], I32, kind="ExternalInput")
    degt = nc.dram_tensor("degw", [P, NT], F32, kind="ExternalInput")
    W = nc.dram_tensor("W", [fdim, odim], F32, kind="ExternalInput")
    b = nc.dram_tensor("b", [1, odim], F32, kind="ExternalInput")
    if pool:
        batt = nc.dram_tensor("batw", [P, NT], F32, kind="ExternalInput")
        iot = nc.dram_tensor("iota64", [P, N_GRAPHS], F32, kind="ExternalInput")
        out = nc.dram_tensor("out", [N_GRAPHS, OUT_DIM], F32,
                             kind="ExternalOutput")
        ar_in = nc.dram_tensor("ar_in", [N_GRAPHS, N_GRAPHS + 1], F32)
        ar_out = nc.dram_tensor("ar_out", [N_GRAPHS, N_GRAPHS + 1], F32,
                                addr_space="Shared")
    else:
        Tout = nc.dram_tensor("Tnext", [R_TOT, odim], BF16,
                              kind="ExternalOutput")
        Tn_in = nc.dram_tensor("Tnext_in", [OWN_PAD, odim], BF16)
        Tn_g = nc.dram_tensor("Tnext_g", [R_TOT, odim], BF16,
                              addr_space="Shared")

    with tile.TileContext(nc) as tc:
        with (
            tc.tile_pool(name="c", bufs=1) as cp,
            tc.tile_pool(name="m", bufs=3) as mp,
            tc.tile_pool(name="s", bufs=3) as sp,
            tc.tile_pool(name="ps", bufs=2, space="PSUM") as pp,
            tc.tile_pool(name="pp2", bufs=1, space="PSUM") as pp2,
        ):
            idxt = cp.tile([P, ktot], I32)
            nc.sync.dma_start(out=idxt[:], in_=idxd[:])
            dw = cp.tile([P, NT], F32)
            nc.sync.dma_start(out=dw[:], in_=degt[:])
            dinv = cp.tile([P, NT], F32)
            nc.scalar.sqrt(dinv[:], dw[:])
            nc.vector.reciprocal(dinv[:], dinv[:])
            Wt = cp.tile([fdim, odim], F32)
            nc.sync.dma_start(out=Wt[:], in_=W[:])
            ident = cp.tile([P, P], F32)
            make_identity(nc, ident[:])
            ones_full = cp.tile([P, P], F32)
            nc.vector.memset(ones_full[:], 1.0)
            brow = cp.tile([P, odim], F32)
            nc.sync.dma_start(out=brow[0:1, :], in_=b[:])
            bp_ps = pp.tile([P, odim], F32, tag="bb")
            nc.tensor.matmul(bp_ps[:], ones_full[0:1, :], brow[0:1, :],
                             start=True, stop=True)
            biasb = cp.tile([P, odim], F32)
            nc.scalar.copy(biasb[:], bp_ps[:])
            if pool:
                batsb = cp.tile([P, NT], F32)
                nc.sync.dma_start(out=batsb[:], in_=batt[:])
                iosb = cp.tile([P, N_GRAPHS], F32)
                nc.sync.dma_start(out=iosb[:], in_=iot[:])
                pool_ps = pp2.tile([N_GRAPHS, N_GRAPHS + 1], F32, tag="pool")

            for t in range(NT):
                km, ko = int(kmax[t]), int(koff[t])
                mt = mp.tile([P, km_cap, fdim], BF16, tag="m")
                for i in range(km):
                    nc.gpsimd.indirect_dma_start(
                        out=mt[:, i, :], out_offset=None, in_=Tin[:],
                        in_offset=bass.IndirectOffsetOnAxis(
                            ap=idxt[:, ko + i:ko + i + 1], axis=0))
                agg = sp.tile([P, fdim], F32, tag="agg")
                nc.vector.tensor_reduce(
                    out=agg[:], in_=mt[:, :km, :].rearrange("p k f -> p f k"),
                    axis=mybir.AxisListType.X, op=mybir.AluOpType.add)
                tp_ps = pp.tile([P, P], F32, tag="tp")
                nc.tensor.transpose(out=tp_ps[:], in_=agg[:],
                                    identity=ident[:])
                aggT = sp.tile([P, P], F32, tag="at")
                nc.scalar.copy(aggT[:], tp_ps[:])
                z_ps = pp.tile([P, odim], F32, tag="z")
                nc.tensor.matmul(z_ps[:], aggT[:], Wt[:], start=True,
                                 stop=True)
                if pool:
                    hn = sp.tile([P, odim + 1], F32, tag="hn")
                    nc.vector.memset(hn[:, odim:odim + 1], 1.0)
                    nc.vector.scalar_tensor_tensor(
                        out=hn[:, :odim], in0=z_ps[:],
                        scalar=dinv[:, t:t + 1], in1=biasb[:],
                        op0=mybir.AluOpType.mult, op1=mybir.AluOpType.add)
                    nc.vector.tensor_relu(out=hn[:, :odim], in_=hn[:, :odim])
                    oh = sp.tile([P, N_GRAPHS], F32, tag="oh")
                    nc.vector.tensor_scalar(
                        out=oh[:], in0=iosb[:], scalar1=batsb[:, t:t + 1],
                        scalar2=None, op0=mybir.AluOpType.is_equal)
                    nc.tensor.matmul(pool_ps[:], oh[:], hn[:],
                                     start=(t == 0), stop=(t == NT - 1))
                else:
                    h = sp.tile([P, odim], F32, tag="h")
                    nc.vector.scalar_tensor_tensor(
                        out=h[:], in0=z_ps[:], scalar=dinv[:, t:t + 1],
                        in1=biasb[:], op0=mybir.AluOpType.mult,
                        op1=mybir.AluOpType.add)
                    nc.vector.tensor_relu(out=h[:], in_=h[:])
                    hs = sp.tile([P, odim], BF16, tag="hs")
                    # dinv=0 on pad rows zeroes them exactly for the dummies
                    nc.scalar.activation(hs[:], h[:], Copy, bias=0.0,
                                         scale=dinv[:, t:t + 1])
                    nc.sync.dma_start(out=Tn_in[t * P:(t + 1) * P, :],
                                      in_=hs[:])

            if pool:
                pool_sb = cp.tile([N_GRAPHS, N_GRAPHS + 1], F32)
                nc.scalar.copy(pool_sb[:], pool_ps[:])
                nc.gpsimd.dma_start(out=ar_in[:], in_=pool_sb[:])
                nc.gpsimd.collective_compute(
                    "AllReduce", mybir.AluOpType.add,
                    replica_groups=[list(range(NCORES))],
                    ins=[ar_in[:]], outs=[ar_out[:]])
                red = cp.tile([N_GRAPHS, N_GRAPHS + 1], F32)
                nc.sync.dma_start(out=red[:], in_=ar_out[:])
                cnt = cp.tile([N_GRAPHS, 1], F32)
                nc.vector.tensor_scalar_max(
                    out=cnt[:], in0=red[:, N_GRAPHS:N_GRAPHS + 1], scalar1=1.0)
                nc.vector.reciprocal(cnt[:], cnt[:])
                res = cp.tile([N_GRAPHS, OUT_DIM], F32)
                nc.scalar.activation(res[:], red[:, :OUT_DIM], Copy,
                                     bias=0.0, scale=cnt[:])
                nc.sync.dma_start(out=out[:], in_=res[:])
            else:
                nc.gpsimd.collective_compute(
                    "AllGather", mybir.AluOpType.bypass,
                    replica_groups=[list(range(NCORES))],
                    ins=[Tn_in[:]], outs=[Tn_g[:]])
                nc.sync.dma_start(out=Tout[:], in_=Tn_g[:])
    nc.compile()
    return nc


# ------------------------------------------------------------ cached runners
def _make_runner(nc, mesh, sh):
    part_name = nc.partition_id_tensor.name if nc.partition_id_tensor else None
    in_names, out_names, out_avals, zero_shapes = [], [], [], []
    for alloc in nc.m.functions[0].allocations:
        if not isinstance(alloc, mybir.MemoryLocationSet):
            continue
        name = alloc.memorylocations[0].name
        if alloc.kind == "ExternalInput":
            if name != part_name:
                in_names.append(name)
        elif alloc.kind == "ExternalOutput":
            out_names.append(name)
            shape = tuple(alloc.tensor_shape)
            dt = mybir.dt.np(alloc.dtype)
            out_avals.append(jax.core.ShapedArray(shape, dt))
            zero_shapes.append((shape, dt))
    n_in = len(in_names)
    all_in = tuple(in_names + out_names
                   + ([part_name] if part_name else []))
    out_avals = tuple(out_avals)
    out_names_t = tuple(out_names)

    def _body(*args):
        operands = list(args)
        if part_name is not None:
            operands.append(partition_id_tensor())
        outs = _bass_exec_p.bind(
            *operands, out_avals=out_avals, in_names=all_in,
            out_names=out_names_t, lowering_input_output_aliases=(),
            sim_require_finite=True, sim_require_nnan=True, nc=nc)
        return tuple(outs)

    spec = PartitionSpec("core")
    n_out = len(out_names)
    jitted = jax.jit(
        shard_map(_body, mesh=mesh, in_specs=(spec,) * (n_in + n_out),
                  out_specs=(spec,) * n_out, check_rep=False),
        donate_argnums=tuple(range(n_in, n_in + n_out)), keep_unused=True)

    zjits = []
    for s, d in zero_shapes:
        gs = (NCORES * s[0],) + tuple(s[1:])
        zjits.append(jax.jit(functools.partial(jnp.zeros, gs, d),
                             out_shardings=sh))
    return {"jitted": jitted, "in_names": in_names, "out_names": out_names,
            "zjits": zjits}


def _run(runner, arrays):
    ins = [arrays[n] for n in runner["in_names"]]
    zs = [zj() for zj in runner["zjits"]]
    outs = runner["jitted"](*ins, *zs)
    return dict(zip(runner["out_names"], outs))


def _rep(a):
    """Replicate a per-core array 8x along axis 0 for P('core') sharding."""
    return np.concatenate([a] * NCORES, axis=0)


_state = {}


def _get_state(edge_index, batch):
    ei = np.asarray(edge_index)
    ba = np.asarray(batch)
    key = (int(ei[0, :64].sum()), int(ei[1, -64:].sum()), int(ba[:512].sum()))
    if key in _state:
        return _state[key]
    install_neuronx_cc_hook()
    prep = host_prep(edge_index, batch)
    mesh = Mesh(np.asarray(jax.devices()[:NCORES]), ("core",))
    sh = NamedSharding(mesh, PartitionSpec("core"))
    nc0 = build_stage0()
    nc1 = build_layer(prep, IN_DIM, HID_DIM, pool=False)
    nc2 = build_layer(prep, HID_DIM, OUT_DIM, pool=True)
    st = {
        "prep": prep, "mesh": mesh, "sh": sh,
        "r0": _make_runner(nc0, mesh, sh),
        "r1": _make_runner(nc1, mesh, sh),
        "r2": _make_runner(nc2, mesh, sh),
        "degw_dev": jax.device_put(
            prep["degw"].reshape(NCORES * P, NT), sh),
        "batw_dev": jax.device_put(
            prep["batw"].reshape(NCORES * P, NT), sh),
        "idx_dev": jax.device_put(
            prep["idx"].reshape(NCORES * P, prep["ktot"]), sh),
        "iota_dev": jax.device_put(_rep(prep["iota64"]), sh),
    }
    _state[key] = st
    return st


def run_gcn(x, W1, b1, W2, b2, edge_index, batch, num_graphs, rep=1):
    st = _get_state(edge_index, batch)
    sh = st["sh"]
    xg = stage_x(np.asarray(x, np.float32), st["prep"])
    xdev = jax.device_put(xg, sh)
    w1d = jax.device_put(_rep(np.asarray(W1, np.float32)), sh)
    b1d = jax.device_put(_rep(np.asarray(b1, np.float32).reshape(1, -1)), sh)
    w2d = jax.device_put(_rep(np.asarray(W2, np.float32)), sh)
    b2d = jax.device_put(_rep(np.asarray(b2, np.float32).reshape(1, -1)), sh)

    o0 = _run(st["r0"], {"x": xdev, "degw": st["degw_dev"]})
    o1 = _run(st["r1"], {"Tin": o0["T0"], "idx": st["idx_dev"],
                         "degw": st["degw_dev"], "W": w1d, "b": b1d})
    o2 = _run(st["r2"], {"Tin": o1["Tnext"], "idx": st["idx_dev"],
                         "degw": st["degw_dev"], "W": w2d, "b": b2d,
                         "batw": st["batw_dev"], "iota64": st["iota_dev"]})
    res = np.asarray(o2["out"])
    return res[:int(num_graphs), :].astype(np.float32)


def kernel(x, W1, b1, W2, b2, edge_index, batch, num_graphs):
    return run_gcn(x, W1, b1, W2, b2, edge_index, batch, num_graphs)
